# revision 23
# baseline (speedup 1.0000x reference)
"""Trainium2 Bass kernel for nn_Attention_47132971106602.

Gated MHA layer: proj -> (rmsnorm+rope on q,k) -> causal attention -> *sigmoid(gate)
-> out-proj.  B=4, L=2048, HID=2048, H=16 heads, DH=128.

Sharding (8 cores, one NEFF, SPMD over different data):
  core = 2*b + g  (b = batch 0..3, g = head-group 0..1 -> heads [8g, 8g+8))
Each core computes QKV+gate proj for its 8 heads on its batch, full causal
attention for those heads, and a partial out-proj (contraction over its 1024
head-dims).  The pair of cores holding one batch produce partial sums of the
final (L, HID) output; the host adds the two partials (TP unshard).

Device layouts (all per-core):
  xT      (HID, L)  f16  hidden[b] transposed     -> SBUF [128, 16, 2048]
  wq/wk/wg(HID, 1024) f16 per-head column blocks  -> streamed per head
  wv      (HID, 1024) f16                          -> V phase
  wout    (1024, HID) f16 rows for this core      -> out-proj phase
  Q^T/K^T computed col-major (d on partitions, tokens free); V token-major;
  scores key-major; exp'ed scores E in f16 with 2^-4 prefactor (cancels in
  softmax normalization, keeps E in f16 range); softmax denominator via
  ones-vector matmul; rmsnorm scale and rope folded into the DVE epilogue.
"""

import os
import sys

sys.path.insert(0, "/opt/trn_rl_repo")

import math
import numpy as np

import concourse.bass as bass
import concourse.bass_isa as bass_isa
import concourse.tile as tile
from concourse import bacc
from concourse import mybir
from concourse.bass_utils import run_bass_kernel_spmd

F16 = mybir.dt.float16
F32 = mybir.dt.float32

HID = 2048
H = 16
DH = 128
L = 2048
B = 4
HPC = 8            # heads per core
KC = HID // 128    # 16 contraction chunks
TQ = 512           # token chunk (matmul free dim)
NTQ = L // TQ      # 4
NTK = L // 128     # 16 token/key tiles
THETA = 10000.0
EPS = 1e-5
SCALE = 1.0 / math.sqrt(DH)
EXP_BIAS = -2.7725887222397811  # -4*ln2; cancels in softmax normalization

_NC_CACHE = {}

_ACT_JSON_SRC = (
    "/nix/store/z022hj2nvbm3nwdizlisq4ylc0y7rd6q-python3-3.13.14-env/lib/"
    "python3.13/site-packages/neuronxcc/pwp/pwp_bin_trainium/act_info.json"
)


def _install_act_tables():
    """Reorder act table sets so exp/ln/copy all resolve to one set -- the
    kernel then needs a single ACT_TABLE_LOAD instead of hundreds."""
    import json
    try:
        with open(_ACT_JSON_SRC) as f:
            d = json.load(f)
    except OSError:
        import glob as _g
        cands = _g.glob(
            "/nix/store/*python*env/lib/python3*/site-packages/neuronxcc/pwp/"
            "pwp_bin_trainium/act_info.json"
        )
        if not cands:
            return
        with open(cands[0]) as f:
            d = json.load(f)
    sets = d.get("act_func_sets", [])
    sets.sort(key=lambda s0: 0 if s0.get("name") == "natural_log_exp_and_others" else 1)
    import os as _os
    outdir = "/tmp/kernel_act_tables"
    _os.makedirs(outdir, exist_ok=True)
    # table binaries are referenced relative to the json's directory
    src_dir = _os.path.dirname(_ACT_JSON_SRC)
    out = _os.path.join(outdir, "act_info.json")
    with open(out, "w") as f:
        json.dump(d, f)
    for fn in _os.listdir(src_dir):
        dst = _os.path.join(outdir, fn)
        if fn != "act_info.json" and not _os.path.exists(dst):
            import shutil as _sh
            _sh.copyfile(_os.path.join(src_dir, fn), dst)
    _os.environ["BASS_ACT_ROOT_JSON_PATH"] = out




def _act_raw(nc, out, in_, func, bias=0.0, scale=1.0):
    eng = nc.scalar
    ins = [eng.lower_ap(in_)]
    if isinstance(bias, bass.AP):
        ins.append(eng.lower_ap(bias))
    else:
        ins.append(mybir.ImmediateValue(dtype=mybir.dt.float32, value=float(bias)))
    ins.append(mybir.ImmediateValue(dtype=mybir.dt.float32, value=float(scale)))
    ins.append(mybir.ImmediateValue(dtype=mybir.dt.float32, value=0.0))
    return eng.add_instruction(
        mybir.InstActivation(
            name=nc.get_next_instruction_name(), func=func, ins=ins, outs=[eng.lower_ap(out)]
        )
    )



def build_nc():
    nc = bacc.Bacc("TRN2", target_bir_lowering=False, debug=False)

    xT = nc.dram_tensor("xT", [HID, L], F16, kind="ExternalInput")
    wq = nc.dram_tensor("wq", [HID, HPC * DH], F16, kind="ExternalInput")
    wk = nc.dram_tensor("wk", [HID, HPC * DH], F16, kind="ExternalInput")
    wv = nc.dram_tensor("wv", [HID, HPC * DH], F16, kind="ExternalInput")
    wg = nc.dram_tensor("wg", [HID, HPC * DH], F16, kind="ExternalInput")
    wout = nc.dram_tensor("wout", [HPC * DH, HID], F16, kind="ExternalInput")
    cosT = nc.dram_tensor("cosT", [DH, L], F16, kind="ExternalInput")
    sinT = nc.dram_tensor("sinT", [DH, L], F16, kind="ExternalInput")
    masks = nc.dram_tensor("masks", [4, 128, TQ], F16, kind="ExternalInput")
    y = nc.dram_tensor("y", [L, HID], F32, kind="ExternalOutput")
    rk_dram = nc.dram_tensor("rk_scratch", [HPC, L], F32)

    with tile.TileContext(nc) as tc:
        with tc.tile_pool(name="big", bufs=1) as big:
            xtp = tc.tile_pool(name="xtp", bufs=1)
            xtp_pool = xtp.__enter__()
            xT_sb = xtp_pool.tile([128, KC, L], F16, tag="xT")
            V_sb = big.tile([128, NTK, HPC * DH], F16, tag="V")       # token-major
            gated = big.tile([128, HPC, L], F16, tag="gated")         # col-major
            cos_sb = big.tile([128, L], F16, tag="cos")
            sin_sb = big.tile([128, L], F16, tag="sin")
            mask_sb = big.tile([128, 4, TQ], F16, tag="mask")
            ones_sb = big.tile([128, 1], F16, tag="ones")
            epsb_sb = big.tile([128, 1], F32, tag="epsb")
            eps128_sb = big.tile([128, 1], F32, tag="eps128")
            ebias_sb = big.tile([128, 1], F32, tag="ebias")

            xT_r = xT[:, :].rearrange("(kc k) t -> k kc t", k=128)
            nc.vector.memset(ones_sb, 1.0)
            nc.vector.memset(epsb_sb, EPS)
            nc.vector.memset(eps128_sb, DH * EPS)
            nc.vector.memset(ebias_sb, EXP_BIAS)

            # ---------------- V projection (all heads, token-major) -------------
            with (
                tc.tile_pool(name="wvp", bufs=2) as wvp,
                tc.tile_pool(name="pv", bufs=2, space="PSUM") as pvp,
            ):
                wv_tiles = []
                for vc in range(2):
                    wv_sb = wvp.tile([128, KC, TQ], F16, tag="wv", name=f"wv{vc}")
                    wv_r = wv[:, vc * TQ:(vc + 1) * TQ].rearrange(
                        "(kc k) c -> k kc c", k=128)
                    if vc == 0:
                        # first tile: split across both hwdge queues, then
                        # stream xT in token-chunks so t=0 can start early
                        nc.sync.dma_start(out=wv_sb[:, 0:8, :], in_=wv_r[:, 0:8, :])
                        nc.scalar.dma_start(out=wv_sb[:, 8:16, :], in_=wv_r[:, 8:16, :])
                        for tch in range(NTK):
                            eng = nc.sync if tch % 2 == 0 else nc.scalar
                            tsl = slice(tch * 128, (tch + 1) * 128)
                            eng.dma_start(
                                out=xT_sb[:, :, tsl], in_=xT_r[:, :, tsl])
                        nc.sync.dma_start(out=cos_sb, in_=cosT[:, :])
                        nc.scalar.dma_start(out=sin_sb, in_=sinT[:, :])
                        nc.sync.dma_start(
                            out=mask_sb,
                            in_=masks[:, :, :].rearrange("r j i -> j r i"))
                    else:
                        nc.sync.dma_start(out=wv_sb, in_=wv_r)
                    wv_tiles.append(wv_sb)
                for vc in range(2):
                    wv_sb = wv_tiles[vc]
                    for t in range(NTK):
                        ps = pvp.tile([128, TQ], F32, tag="pv")
                        for kc in range(KC):
                            nc.tensor.matmul(
                                ps,
                                xT_sb[:, kc, t * 128:(t + 1) * 128],
                                wv_sb[:, kc, :],
                                start=(kc == 0),
                                stop=(kc == KC - 1),
                            )
                        nc.vector.tensor_copy(V_sb[:, t, vc * TQ:(vc + 1) * TQ], ps)

            # ---------------- per-head proj + attention -------------------------
            with (
                tc.tile_pool(name="wp", bufs=2) as wp,
                tc.tile_pool(name="wgp", bufs=1) as wgp,
                tc.tile_pool(name="qkp", bufs=2) as qkp,
                tc.tile_pool(name="rp", bufs=2) as rp,
                tc.tile_pool(name="gp", bufs=1) as gp,
                tc.tile_pool(name="sp", bufs=2) as sp,
                tc.tile_pool(name="ep", bufs=3) as epool,
                tc.tile_pool(name="pp", bufs=2, space="PSUM") as pp,
                tc.tile_pool(name="pss", bufs=2, space="PSUM") as pss,
                tc.tile_pool(name="pav", bufs=2, space="PSUM") as pav,
                tc.tile_pool(name="pv1", bufs=2, space="PSUM") as pv1,
            ):

                def proj_head(h):
                    wq_sb = wp.tile([128, KC, DH], F16, tag="wq", name=f"wq{h}")
                    wk_sb = wp.tile([128, KC, DH], F16, tag="wk", name=f"wk{h}")
                    wg_sb = wgp.tile([128, KC, DH], F16, tag="wg", name=f"wg{h}")
                    for w_sb, w_dram in ((wq_sb, wq), (wk_sb, wk), (wg_sb, wg)):
                        nc.sync.dma_start(
                            out=w_sb,
                            in_=w_dram[:, h * DH:(h + 1) * DH].rearrange(
                                "(kc k) c -> k kc c", k=128
                            ),
                        )
                    roped = {}
                    rkcol = rp.tile([128, NTK], F32, tag="rkcol", name=f"rkcol{h}")
                    for tname, w_sb in (("q", wq_sb), ("k", wk_sb)):
                        rt = rp.tile([128, L], F16, tag=f"{tname}r", name=f"{tname}r{h}")
                        for c in range(NTQ):
                            cs = slice(c * TQ, (c + 1) * TQ)
                            ps = pp.tile([128, TQ], F32, tag="mm")
                            for kc in range(KC):
                                nc.tensor.matmul(
                                    ps,
                                    w_sb[:, kc, :],
                                    xT_sb[:, kc, cs],
                                    start=(kc == 0),
                                    stop=(kc == KC - 1),
                                )
                            qt = qkp.tile([128, TQ], F16, tag="qt")
                            nc.vector.tensor_copy(qt, ps)
                            qsw = qkp.tile([128, TQ], F16, tag="qsw")
                            nc.sync.dma_start(out=qsw[0:64, :], in_=qt[64:128, :])
                            nc.sync.dma_start(out=qsw[64:128, :], in_=qt[0:64, :])
                            # rms: sum_d q^2 via ones-matmul
                            sq = sp.tile([128, TQ], F16, tag="sq")
                            nc.vector.tensor_mul(sq, qt, qt)
                            ms = pv1.tile([1, TQ], F32, tag="v1")
                            nc.tensor.matmul(ms, ones_sb, sq, start=True, stop=True)
                            rn2 = sp.tile([1, TQ], F32, tag="rn2")
                            if tname == "q":
                                # rn_q = 1/sqrt(mean(q^2)+eps)
                                _act_raw(nc, rn2, ms,
                                         mybir.ActivationFunctionType.Rsqrt,
                                         bias=epsb_sb[:1, :], scale=1.0 / DH)
                            else:
                                # rn_k' = 1/sqrt(DH*(mean(k^2)+eps)) = rn_k/sqrt(DH)
                                _act_raw(nc, rn2, ms,
                                         mybir.ActivationFunctionType.Rsqrt,
                                         bias=eps128_sb[:1, :], scale=1.0)
                            # rope halves: t1 = qt*cos ; qsw *= sin_signed
                            t1 = sp.tile([128, TQ], F16, tag="t1")
                            nc.vector.tensor_mul(t1, qt, cos_sb[:, cs])
                            nc.vector.tensor_mul(qsw, qsw, sin_sb[:, cs])
                            if tname == "q":
                                rnb = sp.tile([128, TQ], F32, tag="arr")
                                nc.gpsimd.partition_broadcast(rnb, rn2)
                                nc.vector.tensor_add(t1, t1, qsw)
                                nc.vector.tensor_mul(rt[:, cs], t1, rnb)
                            else:
                                # k-side rms scale rides the softmax-exp scale AP
                                nc.vector.tensor_add(rt[:, cs], t1, qsw)
                                nc.sync.dma_start(
                                    out=rk_dram[h, cs], in_=rn2[0:1, :])
                                nc.sync.dma_start(
                                    out=rkcol[:, c * 4:(c + 1) * 4],
                                    in_=rk_dram[h, cs].rearrange(
                                        "(r j) -> j r", j=128),
                                )
                        roped[tname] = rt
                    # gate
                    gt = gp.tile([128, L], F16, tag="gt", name=f"g{h}")
                    for c in range(NTQ):
                        cs = slice(c * TQ, (c + 1) * TQ)
                        ps = pp.tile([128, TQ], F32, tag="mm")
                        for kc in range(KC):
                            nc.tensor.matmul(
                                ps,
                                wg_sb[:, kc, :],
                                xT_sb[:, kc, cs],
                                start=(kc == 0),
                                stop=(kc == KC - 1),
                            )
                        # sigmoid(g) = 0.5*tanh(g/2)+0.5; tanh shares exp's table set,
                        # the affine part is folded into the epilogue
                        nc.scalar.activation(
                            gt[:, cs], ps, mybir.ActivationFunctionType.Tanh,
                            scale=0.5, bias=0.0)
                    return roped["q"], roped["k"], gt, rkcol

                def attn_head(h, qr, kr, gt, rkcol):
                    for c in range(NTQ):
                        cs = slice(c * TQ, (c + 1) * TQ)
                        nkt = 4 * c + 4
                        pavt = pav.tile([128, TQ], F32, tag="av")
                        esum = sp.tile([128, TQ], F16, tag="esum", name=f"es{h}_{c}")
                        pending = None
                        for kt in range(nkt):
                            r = kt - 4 * c
                            co = max(0, 128 * r)   # masked-out column prefix
                            ncs = slice(c * TQ + co, (c + 1) * TQ)
                            pst = pss.tile([128, TQ], F32, tag="s")
                            nc.tensor.matmul(
                                pst[:, co:],
                                kr[:, kt * 128:(kt + 1) * 128],
                                qr[:, ncs],
                                start=True,
                                stop=True,
                            )
                            e = epool.tile([128, TQ], F16, tag="e")
                            nc.scalar.activation(
                                e[:, co:], pst[:, co:],
                                mybir.ActivationFunctionType.Exp,
                                scale=rkcol[:, kt:kt + 1], bias=ebias_sb[:, :],
                            )
                            if r >= 0:
                                nc.vector.tensor_mul(
                                    e[:, co:], e[:, co:], mask_sb[:, r, co:])
                            if kt == 0:
                                nc.vector.tensor_copy(esum, e)
                            else:
                                nc.vector.tensor_add(
                                    esum[:, co:], esum[:, co:], e[:, co:])
                            if pending is not None:
                                pkt, pe, pco = pending
                                nc.tensor.matmul(
                                    pavt[:, pco:],
                                    V_sb[:, pkt, h * DH:(h + 1) * DH],
                                    pe[:, pco:],
                                    start=(pkt == 0), stop=False,
                                )
                            pending = (kt, e, co)
                        pkt, pe, pco = pending
                        nc.tensor.matmul(
                            pavt[:, pco:], V_sb[:, pkt, h * DH:(h + 1) * DH],
                            pe[:, pco:],
                            start=(pkt == 0), stop=True,
                        )
                        den = pv1.tile([1, TQ], F32, tag="v1", name=f"dn{h}_{c}")
                        nc.tensor.matmul(den, ones_sb, esum, start=True, stop=True)
                        rd = sp.tile([1, TQ], F32, tag="rn2", name=f"rd{h}_{c}")
                        nc.vector.reciprocal_approx_fast(out=rd, in_=den)
                        rdb = sp.tile([128, TQ], F32, tag="arr", name=f"rdb{h}_{c}")
                        nc.gpsimd.partition_broadcast(rdb, rd)
                        tn = sp.tile([128, TQ], F16, tag="tn")
                        # tn = 0.5 * av / den   (0.5 from the tanh half-form)
                        nc.vector.scalar_tensor_tensor(
                            out=tn, in0=pavt, scalar=0.5, in1=rdb,
                            op0=mybir.AluOpType.mult, op1=mybir.AluOpType.mult)
                        # gated = tn * (tanh(g/2) + 1) = av/den * sigmoid(g)
                        nc.vector.scalar_tensor_tensor(
                            out=gated[:, h, cs], in0=gt[:, cs], scalar=1.0,
                            in1=tn,
                            op0=mybir.AluOpType.add, op1=mybir.AluOpType.mult)

                prev = None
                for h in range(HPC):
                    cur = proj_head(h)
                    if prev is not None:
                        attn_head(h - 1, *prev)
                    prev = cur
                attn_head(HPC - 1, *prev)

            xtp.__exit__(None, None, None)

            # ---------------- out projection ------------------------------------
            with (
                tc.tile_pool(name="wo", bufs=1) as wo,
                tc.tile_pool(name="yp", bufs=2) as yp,
                tc.tile_pool(name="py", bufs=2, space="PSUM") as pyp,
            ):
                wout_sb = wo.tile([128, HPC, HID], F16, tag="wout")
                wout_r = wout[:, :].rearrange("(hc c) o -> c hc o", c=128)
                for oc in range(NTQ):
                    ocs = slice(oc * TQ, (oc + 1) * TQ)
                    nc.sync.dma_start(out=wout_sb[:, :, ocs], in_=wout_r[:, :, ocs])
                for t in range(NTK):
                    ysb = yp.tile([128, HID], F32, tag="y")
                    for oc in range(NTQ):
                        ps = pyp.tile([128, TQ], F32, tag="ym")
                        for hc in range(HPC):
                            nc.tensor.matmul(
                                ps,
                                gated[:, hc, t * 128:(t + 1) * 128],
                                wout_sb[:, hc, oc * TQ:(oc + 1) * TQ],
                                start=(hc == 0),
                                stop=(hc == HPC - 1),
                            )
                        nc.vector.tensor_copy(ysb[:, oc * TQ:(oc + 1) * TQ], ps)
                    nc.sync.dma_start(out=y[t * 128:(t + 1) * 128, :], in_=ysb)

    nc.compile()
    return nc


def _host_tables():
    half = DH // 2
    inv_freq = 1.0 / (THETA ** (np.arange(half, dtype=np.float64) * 2.0 / DH))
    pos = np.arange(L, dtype=np.float64)
    ang = pos[:, None] * inv_freq[None, :]          # (L, 64)
    cos = np.cos(ang).T                             # (64, L)
    sin = np.sin(ang).T
    cosT = np.concatenate([cos, cos], axis=0).astype(np.float16)        # (128, L)
    sinT = np.concatenate([-sin, sin], axis=0).astype(np.float16)
    j = np.arange(128)[None, :, None]
    r = np.arange(4)[:, None, None]
    i = np.arange(TQ)[None, None, :]
    masks = ((128 * r + j) <= i).astype(np.float16)                     # (4,128,512)
    return cosT, sinT, masks


def _run(hidden_states, W_qkvg, W_out, trace=False, trace_cores=None):
    key = "nc"
    if key not in _NC_CACHE:
        _NC_CACHE[key] = build_nc()
    nc = _NC_CACHE[key]

    hidden_states = np.asarray(hidden_states)
    W_qkvg = np.asarray(W_qkvg)
    W_out = np.asarray(W_out)

    cosT, sinT, masks = _host_tables()
    QKV = 3 * H * DH

    in_maps = []
    for core in range(8):
        b, g = divmod(core, 2)
        cols = slice(g * HPC * DH, (g + 1) * HPC * DH)
        in_maps.append({
            "xT": np.ascontiguousarray(hidden_states[b].T).astype(np.float16),
            "wq": W_qkvg[:, 0 * H * DH:1 * H * DH][:, cols].astype(np.float16),
            "wk": W_qkvg[:, 1 * H * DH:2 * H * DH][:, cols].astype(np.float16),
            "wv": W_qkvg[:, 2 * H * DH:3 * H * DH][:, cols].astype(np.float16),
            "wg": W_qkvg[:, QKV:][:, cols].astype(np.float16),
            "wout": W_out[cols, :].astype(np.float16),
            "cosT": cosT,
            "sinT": sinT,
            "masks": masks,
        })

    kw = {}
    if trace:
        kw["trace"] = True
        if trace_cores is not None:
            kw["trace_cores"] = trace_cores
    res = run_bass_kernel_spmd(nc, in_maps, core_ids=list(range(8)), **kw)

    out = np.empty((B, L, HID), dtype=np.float32)
    for b in range(B):
        out[b] = res.results[2 * b]["y"] + res.results[2 * b + 1]["y"]
    return out, res


def kernel(hidden_states, W_qkvg, W_out):
    trace = os.environ.get("KERNEL_TRACE", "0") == "1"
    out, res = _run(hidden_states, W_qkvg, W_out, trace=trace)
    kernel.last_results = res
    return out


if __name__ == "__main__":
    rng = np.random.default_rng(0)
    hs = rng.standard_normal((B, L, HID), dtype=np.float32)
    wqkvg = (rng.standard_normal((HID, QKV_ := 3 * H * DH + HID), dtype=np.float32) * 0.02)
    wout = (rng.standard_normal((HID, HID), dtype=np.float32) * 0.02)
    out = kernel(hs, wqkvg, wout)
    print(out.shape, out.dtype)


# revision 24
# speedup vs baseline: 1.0249x; 1.0249x over previous
"""Trainium2 Bass kernel for nn_Attention_47132971106602.

Gated MHA layer: proj -> (rmsnorm+rope on q,k) -> causal attention -> *sigmoid(gate)
-> out-proj.  B=4, L=2048, HID=2048, H=16 heads, DH=128.

Sharding (8 cores, one NEFF, SPMD over different data):
  core = 2*b + g  (b = batch 0..3, g = head-group 0..1 -> heads [8g, 8g+8))
Each core computes QKV+gate proj for its 8 heads on its batch, full causal
attention for those heads, and a partial out-proj (contraction over its 1024
head-dims).  The pair of cores holding one batch produce partial sums of the
final (L, HID) output; the host adds the two partials (TP unshard).

Device layouts (all per-core):
  xT      (HID, L)  f16  hidden[b] transposed     -> SBUF [128, 16, 2048]
  wq/wk/wg(HID, 1024) f16 per-head column blocks  -> streamed per head
  wv      (HID, 1024) f16                          -> V phase
  wout    (1024, HID) f16 rows for this core      -> out-proj phase
  Q^T/K^T computed col-major (d on partitions, tokens free); V token-major;
  scores key-major; exp'ed scores E in f16 with 2^-4 prefactor (cancels in
  softmax normalization, keeps E in f16 range); softmax denominator via
  ones-vector matmul; rmsnorm scale and rope folded into the DVE epilogue.
"""

import os
import sys

sys.path.insert(0, "/opt/trn_rl_repo")

import math
import numpy as np

import concourse.bass as bass
import concourse.bass_isa as bass_isa
import concourse.tile as tile
from concourse import bacc
from concourse import mybir
from concourse.bass_utils import run_bass_kernel_spmd

F16 = mybir.dt.float16
F32 = mybir.dt.float32

HID = 2048
H = 16
DH = 128
L = 2048
B = 4
HPC = 8            # heads per core
KC = HID // 128    # 16 contraction chunks
TQ = 512           # token chunk (matmul free dim)
NTQ = L // TQ      # 4
NTK = L // 128     # 16 token/key tiles
THETA = 10000.0
EPS = 1e-5
SCALE = 1.0 / math.sqrt(DH)
EXP_BIAS = -2.7725887222397811  # -4*ln2; cancels in softmax normalization

_NC_CACHE = {}

_ACT_JSON_SRC = (
    "/nix/store/z022hj2nvbm3nwdizlisq4ylc0y7rd6q-python3-3.13.14-env/lib/"
    "python3.13/site-packages/neuronxcc/pwp/pwp_bin_trainium/act_info.json"
)


def _install_act_tables():
    """Reorder act table sets so exp/ln/copy all resolve to one set -- the
    kernel then needs a single ACT_TABLE_LOAD instead of hundreds."""
    import json
    try:
        with open(_ACT_JSON_SRC) as f:
            d = json.load(f)
    except OSError:
        import glob as _g
        cands = _g.glob(
            "/nix/store/*python*env/lib/python3*/site-packages/neuronxcc/pwp/"
            "pwp_bin_trainium/act_info.json"
        )
        if not cands:
            return
        with open(cands[0]) as f:
            d = json.load(f)
    sets = d.get("act_func_sets", [])
    sets.sort(key=lambda s0: 0 if s0.get("name") == "natural_log_exp_and_others" else 1)
    import os as _os
    outdir = "/tmp/kernel_act_tables"
    _os.makedirs(outdir, exist_ok=True)
    # table binaries are referenced relative to the json's directory
    src_dir = _os.path.dirname(_ACT_JSON_SRC)
    out = _os.path.join(outdir, "act_info.json")
    with open(out, "w") as f:
        json.dump(d, f)
    for fn in _os.listdir(src_dir):
        dst = _os.path.join(outdir, fn)
        if fn != "act_info.json" and not _os.path.exists(dst):
            import shutil as _sh
            _sh.copyfile(_os.path.join(src_dir, fn), dst)
    _os.environ["BASS_ACT_ROOT_JSON_PATH"] = out




def _act_raw(nc, out, in_, func, bias=0.0, scale=1.0):
    eng = nc.scalar
    ins = [eng.lower_ap(in_)]
    if isinstance(bias, bass.AP):
        ins.append(eng.lower_ap(bias))
    else:
        ins.append(mybir.ImmediateValue(dtype=mybir.dt.float32, value=float(bias)))
    ins.append(mybir.ImmediateValue(dtype=mybir.dt.float32, value=float(scale)))
    ins.append(mybir.ImmediateValue(dtype=mybir.dt.float32, value=0.0))
    return eng.add_instruction(
        mybir.InstActivation(
            name=nc.get_next_instruction_name(), func=func, ins=ins, outs=[eng.lower_ap(out)]
        )
    )



def build_nc():
    nc = bacc.Bacc("TRN2", target_bir_lowering=False, debug=False)

    xT = nc.dram_tensor("xT", [HID, L], F16, kind="ExternalInput")
    wq = nc.dram_tensor("wq", [HID, HPC * DH], F16, kind="ExternalInput")
    wk = nc.dram_tensor("wk", [HID, HPC * DH], F16, kind="ExternalInput")
    wv = nc.dram_tensor("wv", [HID, HPC * DH], F16, kind="ExternalInput")
    wg = nc.dram_tensor("wg", [HID, HPC * DH], F16, kind="ExternalInput")
    wout = nc.dram_tensor("wout", [HPC * DH, HID], F16, kind="ExternalInput")
    cosT = nc.dram_tensor("cosT", [DH, L], F16, kind="ExternalInput")
    sinT = nc.dram_tensor("sinT", [DH, L], F16, kind="ExternalInput")
    masks = nc.dram_tensor("masks", [4, 128, TQ], F16, kind="ExternalInput")
    y = nc.dram_tensor("y", [L, HID], F32, kind="ExternalOutput")
    rk_dram = nc.dram_tensor("rk_scratch", [HPC, L], F32)

    with tile.TileContext(nc) as tc:
        with tc.tile_pool(name="big", bufs=1) as big:
            xtp = tc.tile_pool(name="xtp", bufs=1)
            xtp_pool = xtp.__enter__()
            xT_sb = xtp_pool.tile([128, KC, L], F16, tag="xT")
            V_sb = big.tile([128, NTK, HPC * DH], F16, tag="V")       # token-major
            gated = big.tile([128, HPC, L], F16, tag="gated")         # col-major
            cos_sb = big.tile([128, L], F16, tag="cos")
            sin_sb = big.tile([128, L], F16, tag="sin")
            mask_sb = big.tile([128, 4, TQ], F16, tag="mask")
            ones_sb = big.tile([128, 1], F16, tag="ones")
            epsb_sb = big.tile([128, 1], F32, tag="epsb")
            eps128_sb = big.tile([128, 1], F32, tag="eps128")
            ebias_sb = big.tile([128, 1], F32, tag="ebias")

            xT_r = xT[:, :].rearrange("(kc k) t -> k kc t", k=128)
            nc.vector.memset(ones_sb, 1.0)
            nc.vector.memset(epsb_sb, EPS)
            nc.vector.memset(eps128_sb, DH * EPS)
            nc.vector.memset(ebias_sb, EXP_BIAS)

            # ---------------- V projection (all heads, token-major) -------------
            with (
                tc.tile_pool(name="wvp", bufs=2) as wvp,
                tc.tile_pool(name="pv", bufs=2, space="PSUM") as pvp,
            ):
                wv_tiles = []
                for vc in range(2):
                    wv_sb = wvp.tile([128, KC, TQ], F16, tag="wv", name=f"wv{vc}")
                    wv_r = wv[:, vc * TQ:(vc + 1) * TQ].rearrange(
                        "(kc k) c -> k kc c", k=128)
                    if vc == 0:
                        # first tile: split across both hwdge queues, then
                        # stream xT in token-chunks so t=0 can start early
                        nc.sync.dma_start(out=wv_sb[:, 0:8, :], in_=wv_r[:, 0:8, :])
                        nc.scalar.dma_start(out=wv_sb[:, 8:16, :], in_=wv_r[:, 8:16, :])
                        for tch in range(NTK):
                            eng = nc.sync if tch % 2 == 0 else nc.scalar
                            tsl = slice(tch * 128, (tch + 1) * 128)
                            eng.dma_start(
                                out=xT_sb[:, :, tsl], in_=xT_r[:, :, tsl])
                        nc.sync.dma_start(out=cos_sb, in_=cosT[:, :])
                        nc.scalar.dma_start(out=sin_sb, in_=sinT[:, :])
                        nc.sync.dma_start(
                            out=mask_sb,
                            in_=masks[:, :, :].rearrange("r j i -> j r i"))
                    else:
                        nc.sync.dma_start(out=wv_sb, in_=wv_r)
                    wv_tiles.append(wv_sb)
                for vc in range(2):
                    wv_sb = wv_tiles[vc]
                    for t in range(NTK):
                        ps = pvp.tile([128, TQ], F32, tag="pv")
                        for kc in range(KC):
                            nc.tensor.matmul(
                                ps,
                                xT_sb[:, kc, t * 128:(t + 1) * 128],
                                wv_sb[:, kc, :],
                                start=(kc == 0),
                                stop=(kc == KC - 1),
                            )
                        nc.vector.tensor_copy(V_sb[:, t, vc * TQ:(vc + 1) * TQ], ps)

            # ---------------- per-head proj + attention -------------------------
            with (
                tc.tile_pool(name="wp", bufs=2) as wp,
                tc.tile_pool(name="wgp", bufs=1) as wgp,
                tc.tile_pool(name="qkp", bufs=2) as qkp,
                tc.tile_pool(name="rp", bufs=2) as rp,
                tc.tile_pool(name="gp", bufs=1) as gp,
                tc.tile_pool(name="sp", bufs=2) as sp,
                tc.tile_pool(name="ep", bufs=2) as epool,
                tc.tile_pool(name="pp", bufs=2, space="PSUM") as pp,
                tc.tile_pool(name="pss", bufs=2, space="PSUM") as pss,
                tc.tile_pool(name="pav", bufs=2, space="PSUM") as pav,
                tc.tile_pool(name="pv1", bufs=2, space="PSUM") as pv1,
            ):

                def proj_head(h):
                    wq_sb = wp.tile([128, KC, DH], F16, tag="wq", name=f"wq{h}")
                    wk_sb = wp.tile([128, KC, DH], F16, tag="wk", name=f"wk{h}")
                    wg_sb = wgp.tile([128, KC, DH], F16, tag="wg", name=f"wg{h}")
                    for w_sb, w_dram in ((wq_sb, wq), (wk_sb, wk), (wg_sb, wg)):
                        nc.sync.dma_start(
                            out=w_sb,
                            in_=w_dram[:, h * DH:(h + 1) * DH].rearrange(
                                "(kc k) c -> k kc c", k=128
                            ),
                        )
                    roped = {}
                    rkcol = rp.tile([128, NTK], F32, tag="rkcol", name=f"rkcol{h}")
                    for tname, w_sb in (("q", wq_sb), ("k", wk_sb)):
                        rt = rp.tile([128, L], F16, tag=f"{tname}r", name=f"{tname}r{h}")
                        for c in range(NTQ):
                            cs = slice(c * TQ, (c + 1) * TQ)
                            ps = pp.tile([128, TQ], F32, tag="mm")
                            for kc in range(KC):
                                nc.tensor.matmul(
                                    ps,
                                    w_sb[:, kc, :],
                                    xT_sb[:, kc, cs],
                                    start=(kc == 0),
                                    stop=(kc == KC - 1),
                                )
                            qt = qkp.tile([128, TQ], F16, tag="qt")
                            nc.vector.tensor_copy(qt, ps)
                            qsw = qkp.tile([128, TQ], F16, tag="qsw")
                            nc.sync.dma_start(out=qsw[0:64, :], in_=qt[64:128, :])
                            nc.sync.dma_start(out=qsw[64:128, :], in_=qt[0:64, :])
                            # rms: sum_d q^2 via ones-matmul
                            sq = sp.tile([128, TQ], F16, tag="sq")
                            nc.vector.tensor_mul(sq, qt, qt)
                            ms = pv1.tile([1, TQ], F32, tag="v1")
                            nc.tensor.matmul(ms, ones_sb, sq, start=True, stop=True)
                            rn2 = sp.tile([1, TQ], F32, tag="rn2")
                            if tname == "q":
                                # rn_q = 1/sqrt(mean(q^2)+eps)
                                _act_raw(nc, rn2, ms,
                                         mybir.ActivationFunctionType.Rsqrt,
                                         bias=epsb_sb[:1, :], scale=1.0 / DH)
                            else:
                                # rn_k' = 1/sqrt(DH*(mean(k^2)+eps)) = rn_k/sqrt(DH)
                                _act_raw(nc, rn2, ms,
                                         mybir.ActivationFunctionType.Rsqrt,
                                         bias=eps128_sb[:1, :], scale=1.0)
                            # rope halves: t1 = qt*cos ; qsw *= sin_signed
                            t1 = sp.tile([128, TQ], F16, tag="t1")
                            nc.vector.tensor_mul(t1, qt, cos_sb[:, cs])
                            nc.vector.tensor_mul(qsw, qsw, sin_sb[:, cs])
                            if tname == "q":
                                rnb = sp.tile([128, TQ], F32, tag="arr")
                                nc.gpsimd.partition_broadcast(rnb, rn2)
                                nc.vector.tensor_add(t1, t1, qsw)
                                nc.vector.tensor_mul(rt[:, cs], t1, rnb)
                            else:
                                # k-side rms scale rides the softmax-exp scale AP
                                nc.vector.tensor_add(rt[:, cs], t1, qsw)
                                nc.sync.dma_start(
                                    out=rk_dram[h, cs], in_=rn2[0:1, :])
                                nc.sync.dma_start(
                                    out=rkcol[:, c * 4:(c + 1) * 4],
                                    in_=rk_dram[h, cs].rearrange(
                                        "(r j) -> j r", j=128),
                                )
                        roped[tname] = rt
                    # gate
                    gt = gp.tile([128, L], F16, tag="gt", name=f"g{h}")
                    for c in range(NTQ):
                        cs = slice(c * TQ, (c + 1) * TQ)
                        ps = pp.tile([128, TQ], F32, tag="mm")
                        for kc in range(KC):
                            nc.tensor.matmul(
                                ps,
                                wg_sb[:, kc, :],
                                xT_sb[:, kc, cs],
                                start=(kc == 0),
                                stop=(kc == KC - 1),
                            )
                        # sigmoid(g) = 0.5*tanh(g/2)+0.5; tanh shares exp's table set,
                        # the affine part is folded into the epilogue
                        nc.scalar.activation(
                            gt[:, cs], ps, mybir.ActivationFunctionType.Tanh,
                            scale=0.5, bias=0.0)
                    return roped["q"], roped["k"], gt, rkcol

                def attn_head(h, qr, kr, gt, rkcol):
                    for c in range(NTQ):
                        cs = slice(c * TQ, (c + 1) * TQ)
                        nkt = 4 * c + 4
                        pavt = pav.tile([128, TQ], F32, tag="av")
                        esum = sp.tile([128, TQ], F16, tag="esum", name=f"es{h}_{c}")
                        pending = None
                        for kt in range(nkt):
                            r = kt - 4 * c
                            co = max(0, 128 * r)   # masked-out column prefix
                            ncs = slice(c * TQ + co, (c + 1) * TQ)
                            pst = pss.tile([128, TQ], F32, tag="s")
                            nc.tensor.matmul(
                                pst[:, co:],
                                kr[:, kt * 128:(kt + 1) * 128],
                                qr[:, ncs],
                                start=True,
                                stop=True,
                            )
                            e = epool.tile([128, TQ], F16, tag="e")
                            nc.scalar.activation(
                                e[:, co:], pst[:, co:],
                                mybir.ActivationFunctionType.Exp,
                                scale=rkcol[:, kt:kt + 1], bias=ebias_sb[:, :],
                            )
                            if r >= 0:
                                nc.vector.tensor_mul(
                                    e[:, co:], e[:, co:], mask_sb[:, r, co:])
                            if kt == 0:
                                nc.vector.tensor_copy(esum, e)
                            else:
                                nc.vector.tensor_add(
                                    esum[:, co:], esum[:, co:], e[:, co:])
                            if pending is not None:
                                pkt, pe, pco = pending
                                nc.tensor.matmul(
                                    pavt[:, pco:],
                                    V_sb[:, pkt, h * DH:(h + 1) * DH],
                                    pe[:, pco:],
                                    start=(pkt == 0), stop=False,
                                )
                            pending = (kt, e, co)
                        pkt, pe, pco = pending
                        nc.tensor.matmul(
                            pavt[:, pco:], V_sb[:, pkt, h * DH:(h + 1) * DH],
                            pe[:, pco:],
                            start=(pkt == 0), stop=True,
                        )
                        den = pv1.tile([1, TQ], F32, tag="v1", name=f"dn{h}_{c}")
                        nc.tensor.matmul(den, ones_sb, esum, start=True, stop=True)
                        rd = sp.tile([1, TQ], F32, tag="rn2", name=f"rd{h}_{c}")
                        nc.vector.reciprocal_approx_fast(out=rd, in_=den)
                        rdb = sp.tile([128, TQ], F32, tag="arr", name=f"rdb{h}_{c}")
                        nc.gpsimd.partition_broadcast(rdb, rd)
                        tn = sp.tile([128, TQ], F16, tag="tn")
                        # tn = 0.5 * av / den   (0.5 from the tanh half-form)
                        nc.vector.scalar_tensor_tensor(
                            out=tn, in0=pavt, scalar=0.5, in1=rdb,
                            op0=mybir.AluOpType.mult, op1=mybir.AluOpType.mult)
                        # gated = tn * (tanh(g/2) + 1) = av/den * sigmoid(g)
                        nc.vector.scalar_tensor_tensor(
                            out=gated[:, h, cs], in0=gt[:, cs], scalar=1.0,
                            in1=tn,
                            op0=mybir.AluOpType.add, op1=mybir.AluOpType.mult)

                prev = None
                for h in range(HPC):
                    cur = proj_head(h)
                    if prev is not None:
                        attn_head(h - 1, *prev)
                    prev = cur
                attn_head(HPC - 1, *prev)

            xtp.__exit__(None, None, None)

            # ---------------- out projection ------------------------------------
            with (
                tc.tile_pool(name="wo", bufs=1) as wo,
                tc.tile_pool(name="yp", bufs=2) as yp,
                tc.tile_pool(name="py", bufs=2, space="PSUM") as pyp,
            ):
                wout_sb = wo.tile([128, HPC, HID], F16, tag="wout")
                wout_r = wout[:, :].rearrange("(hc c) o -> c hc o", c=128)
                for oc in range(NTQ):
                    ocs = slice(oc * TQ, (oc + 1) * TQ)
                    nc.sync.dma_start(out=wout_sb[:, :, ocs], in_=wout_r[:, :, ocs])
                for t in range(NTK):
                    ysb = yp.tile([128, HID], F32, tag="y")
                    for oc in range(NTQ):
                        ps = pyp.tile([128, TQ], F32, tag="ym")
                        for hc in range(HPC):
                            nc.tensor.matmul(
                                ps,
                                gated[:, hc, t * 128:(t + 1) * 128],
                                wout_sb[:, hc, oc * TQ:(oc + 1) * TQ],
                                start=(hc == 0),
                                stop=(hc == HPC - 1),
                            )
                        nc.vector.tensor_copy(ysb[:, oc * TQ:(oc + 1) * TQ], ps)
                    nc.sync.dma_start(out=y[t * 128:(t + 1) * 128, :], in_=ysb)

    nc.compile()
    return nc


def _host_tables():
    half = DH // 2
    inv_freq = 1.0 / (THETA ** (np.arange(half, dtype=np.float64) * 2.0 / DH))
    pos = np.arange(L, dtype=np.float64)
    ang = pos[:, None] * inv_freq[None, :]          # (L, 64)
    cos = np.cos(ang).T                             # (64, L)
    sin = np.sin(ang).T
    cosT = np.concatenate([cos, cos], axis=0).astype(np.float16)        # (128, L)
    sinT = np.concatenate([-sin, sin], axis=0).astype(np.float16)
    j = np.arange(128)[None, :, None]
    r = np.arange(4)[:, None, None]
    i = np.arange(TQ)[None, None, :]
    masks = ((128 * r + j) <= i).astype(np.float16)                     # (4,128,512)
    return cosT, sinT, masks


def _run(hidden_states, W_qkvg, W_out, trace=False, trace_cores=None):
    key = "nc"
    if key not in _NC_CACHE:
        _NC_CACHE[key] = build_nc()
    nc = _NC_CACHE[key]

    hidden_states = np.asarray(hidden_states)
    W_qkvg = np.asarray(W_qkvg)
    W_out = np.asarray(W_out)

    cosT, sinT, masks = _host_tables()
    QKV = 3 * H * DH

    in_maps = []
    for core in range(8):
        b, g = divmod(core, 2)
        cols = slice(g * HPC * DH, (g + 1) * HPC * DH)
        in_maps.append({
            "xT": np.ascontiguousarray(hidden_states[b].T).astype(np.float16),
            "wq": W_qkvg[:, 0 * H * DH:1 * H * DH][:, cols].astype(np.float16),
            "wk": W_qkvg[:, 1 * H * DH:2 * H * DH][:, cols].astype(np.float16),
            "wv": W_qkvg[:, 2 * H * DH:3 * H * DH][:, cols].astype(np.float16),
            "wg": W_qkvg[:, QKV:][:, cols].astype(np.float16),
            "wout": W_out[cols, :].astype(np.float16),
            "cosT": cosT,
            "sinT": sinT,
            "masks": masks,
        })

    kw = {}
    if trace:
        kw["trace"] = True
        if trace_cores is not None:
            kw["trace_cores"] = trace_cores
    res = run_bass_kernel_spmd(nc, in_maps, core_ids=list(range(8)), **kw)

    out = np.empty((B, L, HID), dtype=np.float32)
    for b in range(B):
        out[b] = res.results[2 * b]["y"] + res.results[2 * b + 1]["y"]
    return out, res


def kernel(hidden_states, W_qkvg, W_out):
    trace = os.environ.get("KERNEL_TRACE", "0") == "1"
    out, res = _run(hidden_states, W_qkvg, W_out, trace=trace)
    kernel.last_results = res
    return out


if __name__ == "__main__":
    rng = np.random.default_rng(0)
    hs = rng.standard_normal((B, L, HID), dtype=np.float32)
    wqkvg = (rng.standard_normal((HID, QKV_ := 3 * H * DH + HID), dtype=np.float32) * 0.02)
    wout = (rng.standard_normal((HID, HID), dtype=np.float32) * 0.02)
    out = kernel(hs, wqkvg, wout)
    print(out.shape, out.dtype)
